# revision 2
# baseline (speedup 1.0000x reference)
"""Trainium2 Bass kernel for DWConvBlock3D:
depthwise 3x3x3 conv (pad 1) + InstanceNorm3d + ReLU on x:(2,64,64,128,128) f32.

Strategy (8 NeuronCores, channel sharding => zero communication):
  - Each core owns 8 channels x 2 batches = 16 (b,c) "pairs".
  - Layout per pair: H=128 on SBUF partitions, (D,W) on the free dim, with
    host-side zero padding in both D (66) and W (130) so every matmul is
    uniform (no edge clipping).
  - The conv runs on TensorE as banded matmuls: a 128x128 banded matrix
    (3 diagonals = the kh taps) multiplies a (d,w)-shifted view of the x
    tile; shifts cover (kd,kw).  In fp16 that is 9 matmuls per 512-col
    chunk.  In fp8 mode we use DoubleRow perf mode (2 fused k-tile
    matmuls at 0.5 cycles/col): x is split hi+lo e4m3 (xh+xl ~ fp16
    precision) and the band w into bh+bl; the 28 products
      bh_t(xh+xl) [9], bl_t*xh [9], bl_c*xl [1], rest paired
    fit in 14 DoubleRow matmuls = 3584 PE cycles/chunk vs fp16's 4608.
  - PSUM: 2 bufs of 4 banks; each group of 4 chunks (2048 cols) is evicted
    in ONE ScalarE activation-copy (fp32->fp16, accum_out gives sum(y)).
  - sum(y^2) in one DVE pass (fp16 2x mode); cross-partition reduction via
    GpSimd partition_all_reduce; final normalize+ReLU is a single ScalarE
    activation with per-partition scale/bias; output DMA'd as fp16.
"""

import sys

if "/opt/trn_rl_repo" not in sys.path:
    sys.path.insert(0, "/opt/trn_rl_repo")

import numpy as np

B, C, D, H, W = 2, 64, 64, 128, 128
N_CORES = 8
CH_PER_CORE = C // N_CORES  # 8
N_PAIRS = B * CH_PER_CORE  # 16
DP = D + 2  # host-padded D
WP = W + 2  # host-padded W
FREE = D * W  # 8192 output cols per partition per pair
NV = D * H * W  # normalization element count per (b,c)
EPS = 1e-5
CD = 4  # d-slices per chunk (4*128 = 512 fp32 = 1 PSUM bank)
GROUP = 4  # chunks per PSUM buffer (4 banks, evicted as one 2048-col copy)
N_CHUNKS = D // CD  # 16
N_GROUPS = N_CHUNKS // GROUP  # 4
TAPS = [(kd, kw) for kd in range(3) for kw in range(3)]
SV = 2 * DP * WP  # fp8 x tile: partition stride (v=2 halves)
CONV_MODE = "fp8"  # "fp8" (DoubleRow) or "fp16"


def build_program(mode=CONV_MODE):
    import concourse.bacc as bacc
    import concourse.mybir as mybir
    from concourse import bass_isa
    from concourse.tile import TileContext
    from bass_rust import VecI64Pair

    f32 = mybir.dt.float32
    f16 = mybir.dt.float16
    f8 = mybir.dt.float8e4
    fp8 = mode == "fp8"
    nc = bacc.Bacc("TRN2", target_bir_lowering=False, debug=False, num_devices=N_CORES)

    if fp8:
        xs = nc.dram_tensor("xs", [N_PAIRS, H, 2, DP, WP], f8, kind="ExternalInput")
        bands = nc.dram_tensor(
            "bands", [H, CH_PER_CORE, 14, 2, H], f8, kind="ExternalInput"
        )
    else:
        xs = nc.dram_tensor("xs", [N_PAIRS, H, DP, WP], f16, kind="ExternalInput")
        bands = nc.dram_tensor(
            "bands", [H, CH_PER_CORE, 9, H], f16, kind="ExternalInput"
        )
    gb = nc.dram_tensor("gb", [128, 2 * N_PAIRS], f32, kind="ExternalInput")
    out = nc.dram_tensor("out", [N_PAIRS, H, FREE], f16, kind="ExternalOutput")

    mm_kw = dict(skip_group_check=True)
    if fp8:
        mm_kw["perf_mode"] = mybir.MatmulPerfMode.DoubleRow
        n_j = 14
    else:
        n_j = 9

    def rhs_view(xt, d0, j):
        """Moving operand for matmul j of the chunk at (true) depth d0."""
        if not fp8:
            kd, kw = TAPS[j]
            return xt[:, d0 + kd : d0 + kd + CD, kw : kw + W]
        if j < 9:  # (bh_t, bh_t) . (xh@t, xl@t)
            kd, kw = TAPS[j]
            return xt[:, :, d0 + kd : d0 + kd + CD, kw : kw + W]
        if j == 13:  # (bl_c, bl_c) . (xh@c, xl@c)
            return xt[:, :, d0 + 1 : d0 + 1 + CD, 1 : 1 + W]
        if j < 12:  # (bl_(0,kw), bl_(2,kw)) . (xh@(0,kw), xh@(2,kw))
            kw = j - 9
            delta = 2 * WP
        else:  # j == 12: (bl_(1,0), bl_(1,2)) . (xh@(1,0), xh@(1,2))
            kw = 0
            delta = 2
        kd0 = 0 if j < 12 else 1
        v = xt[:, :, d0 + kd0 : d0 + kd0 + CD, kw : kw + W].copy()
        ap = [tuple(p) for p in v.ap]
        ap[1] = (delta, 2)
        v.ap = VecI64Pair(ap)
        return v

    with TileContext(nc) as tc:
        with (
            tc.tile_pool(name="singles", bufs=1) as singles,
            tc.tile_pool(name="xp", bufs=3) as xpool,
            tc.tile_pool(name="yp", bufs=3) as ypool,
            tc.tile_pool(name="st", bufs=4) as stats,
            tc.tile_pool(name="psmm", bufs=2, space="PSUM") as psum_mm,
        ):
            if fp8:
                band_sb = singles.tile([H, CH_PER_CORE, 14, 2, H], f8)
            else:
                band_sb = singles.tile([H, CH_PER_CORE, 9, H], f16)
            gb_sb = singles.tile([128, 2 * N_PAIRS], f32)
            sq_sb = singles.tile([128, FREE], f16)  # y^2 scratch
            nc.sync.dma_start(out=gb_sb[:], in_=gb[:])

            for p in range(N_PAIRS):
                ci = p % CH_PER_CORE

                if fp8:
                    xt = xpool.tile([H, 2, DP, WP], f8, tag="xt")
                else:
                    xt = xpool.tile([H, DP, WP], f16, tag="xt")
                nc.sync.dma_start(out=xt[:], in_=xs[p])
                if p < CH_PER_CORE:
                    # just-in-time per-channel band load (keeps startup short)
                    nc.sync.dma_start(out=band_sb[:, ci], in_=bands[:, ci])

                y = ypool.tile([H, FREE], f16, tag="y")
                sums = stats.tile([128, N_GROUPS], f32, tag="sums")
                st2 = stats.tile([128, 2], f32, tag="st2")

                # ---- depthwise conv: banded matmuls, PSUM-accumulated
                for g in range(N_GROUPS):
                    ps = psum_mm.tile([128, GROUP * CD * W], f32, tag="mm",
                                      name=f"mm_{p}_{g}")
                    for j in range(n_j):
                        for c in range(GROUP):
                            d0 = (g * GROUP + c) * CD
                            if fp8:
                                lhsT = band_sb[:, ci, j]
                            else:
                                lhsT = band_sb[:, ci, j]
                            nc.tensor.matmul(
                                ps[:, c * CD * W : (c + 1) * CD * W],
                                lhsT,
                                rhs_view(xt, d0, j),
                                start=(j == 0),
                                stop=(j == n_j - 1),
                                **mm_kw,
                            )
                    # ---- evict group: PSUM -> y (fp16), sum(y) via accum_out
                    nc.scalar.activation(
                        out=y[:, g * GROUP * CD * W : (g + 1) * GROUP * CD * W],
                        in_=ps[:],
                        func=mybir.ActivationFunctionType.Copy,
                        accum_out=sums[:, g : g + 1],
                    )

                # ---- per-partition stats
                nc.vector.tensor_reduce(
                    out=st2[:, 0:1], in_=sums[:], axis=mybir.AxisListType.X,
                    op=mybir.AluOpType.add,
                )
                # sum(y^2) in one DVE pass (fp16 2x mode)
                nc.vector.scalar_tensor_tensor(
                    out=sq_sb[:], in0=y[:], scalar=1.0, in1=y[:],
                    op0=mybir.AluOpType.mult, op1=mybir.AluOpType.mult,
                    accum_out=st2[:, 1:2],
                )

                # ---- all-reduce across partitions (GpSimd) -> every partition
                # holds (sum, sumsq); the stats math then runs replicated
                ast = stats.tile([128, 2], f32, tag="ast")
                nc.gpsimd.partition_all_reduce(
                    ast[:], st2[:], 128, bass_isa.ReduceOp.add
                )

                sm = stats.tile([128, 10], f32, tag="sm")
                mean, ex2 = sm[:, 0:1], sm[:, 1:2]
                msq, vpe = sm[:, 2:3], sm[:, 3:4]
                std, r0 = sm[:, 4:5], sm[:, 5:6]
                t1, t2 = sm[:, 6:7], sm[:, 7:8]
                t4, rr = sm[:, 8:9], sm[:, 9:10]
                nc.vector.tensor_scalar_mul(mean, ast[:, 0:1], 1.0 / NV)
                nc.vector.tensor_scalar_mul(ex2, ast[:, 1:2], 1.0 / NV)
                nc.vector.tensor_mul(msq, mean, mean)
                nc.vector.tensor_sub(vpe, ex2, msq)
                nc.vector.tensor_scalar_add(vpe, vpe, EPS)
                nc.scalar.activation(std, vpe, mybir.ActivationFunctionType.Sqrt)
                nc.vector.reciprocal(r0, std)
                # one Newton step: r = r0*(1.5 - 0.5*vpe*r0^2)
                nc.vector.tensor_mul(t1, r0, r0)
                nc.vector.tensor_mul(t2, t1, vpe)
                nc.vector.tensor_scalar(
                    t4, t2, -0.5, 1.5, op0=mybir.AluOpType.mult, op1=mybir.AluOpType.add
                )
                nc.vector.tensor_mul(rr, r0, t4)

                sb2 = stats.tile([128, 2], f32, tag="sb2")
                sc, bi = sb2[:, 0:1], sb2[:, 1:2]
                # scale = gamma * rstd ; bias = beta - mean*scale
                nc.vector.tensor_mul(sc, rr, gb_sb[:, p : p + 1])
                nc.vector.tensor_mul(t1, mean, sc)
                nc.vector.tensor_sub(bi, gb_sb[:, N_PAIRS + p : N_PAIRS + p + 1], t1)

                # ---- fused normalize + ReLU (in place), then store.
                # split halves so the ScalarE apply overlaps the out-DMA
                hf = FREE // 2
                for h2 in range(2):
                    ysl = y[:, h2 * hf : (h2 + 1) * hf]
                    nc.scalar.activation(
                        out=ysl,
                        in_=ysl,
                        func=mybir.ActivationFunctionType.Relu,
                        scale=sc,
                        bias=bi,
                    )
                    nc.gpsimd.dma_start(
                        out=out[p][:, h2 * hf : (h2 + 1) * hf], in_=ysl
                    )

    nc.compile()
    return nc


_NC_CACHE = None


def _get_program():
    global _NC_CACHE
    if _NC_CACHE is None:
        _NC_CACHE = build_program()
    return _NC_CACHE


def _band_mats(w, c):
    """f32 band matrices [9, H, H] for channel c, tap order TAPS."""
    eye0 = np.eye(H, dtype=np.float32)
    eyep = np.eye(H, k=1, dtype=np.float32)  # B[h-1, h]: kh=0 tap
    eyem = np.eye(H, k=-1, dtype=np.float32)  # B[h+1, h]: kh=2 tap
    mats = np.empty((9, H, H), np.float32)
    for t, (kd, kw) in enumerate(TAPS):
        wk = w[c, 0, kd, :, kw]
        mats[t] = wk[0] * eyep + wk[1] * eye0 + wk[2] * eyem
    return mats


def make_core_inputs(x, w, gamma, beta, core, mode=CONV_MODE):
    import ml_dtypes

    f8 = ml_dtypes.float8_e4m3
    cs = slice(CH_PER_CORE * core, CH_PER_CORE * (core + 1))
    # (b, ci, d, h, w) -> (b, ci, h, d, w) -> (pair, h, d, w), pair = b*8+ci
    xt = (
        np.ascontiguousarray(x[:, cs].transpose(0, 1, 3, 2, 4))
        .reshape(N_PAIRS, H, D, W)
    )
    if mode == "fp8":
        xc = np.zeros((N_PAIRS, H, 2, DP, WP), f8)
        xh = xt.astype(f8)
        xl = (xt - xh.astype(np.float32)).astype(f8)
        xc[:, :, 0, 1 : D + 1, 1 : W + 1] = xh
        xc[:, :, 1, 1 : D + 1, 1 : W + 1] = xl
        bpk = np.zeros((H, CH_PER_CORE, 14, 2, H), np.float32)
        for ci in range(CH_PER_CORE):
            mats = _band_mats(w, CH_PER_CORE * core + ci)
            bh = mats.astype(f8).astype(np.float32)
            bl = (mats - bh).astype(f8).astype(np.float32)
            t_of = {t: i for i, t in enumerate(TAPS)}
            for j in range(9):
                bpk[:, ci, j, 0] = bh[j]
                bpk[:, ci, j, 1] = bh[j]
            for kw in range(3):
                bpk[:, ci, 9 + kw, 0] = bl[t_of[(0, kw)]]
                bpk[:, ci, 9 + kw, 1] = bl[t_of[(2, kw)]]
            bpk[:, ci, 12, 0] = bl[t_of[(1, 0)]]
            bpk[:, ci, 12, 1] = bl[t_of[(1, 2)]]
            bpk[:, ci, 13, 0] = bl[t_of[(1, 1)]]
            bpk[:, ci, 13, 1] = bl[t_of[(1, 1)]]
        band_arr = bpk.astype(f8)
    else:
        xc = np.zeros((N_PAIRS, H, DP, WP), np.float16)
        xc[:, :, 1 : D + 1, 1 : W + 1] = xt.astype(np.float16)
        bpk = np.zeros((H, CH_PER_CORE, 9, H), np.float32)
        for ci in range(CH_PER_CORE):
            bpk[:, ci] = _band_mats(w, CH_PER_CORE * core + ci).transpose(1, 0, 2)
        band_arr = bpk.astype(np.float16)
    gbv = np.broadcast_to(
        np.concatenate([np.tile(gamma[cs], B), np.tile(beta[cs], B)])
        .astype(np.float32)
        .reshape(1, 2 * N_PAIRS),
        (128, 2 * N_PAIRS),
    ).copy()
    return {"xs": xc, "bands": band_arr, "gb": gbv}


def kernel(x, w, gamma, beta):
    from concourse.bass_utils import run_bass_kernel_spmd

    x = np.asarray(x, dtype=np.float32)
    w = np.asarray(w, dtype=np.float32)
    gamma = np.asarray(gamma, dtype=np.float32)
    beta = np.asarray(beta, dtype=np.float32)

    nc = _get_program()
    in_maps = [make_core_inputs(x, w, gamma, beta, k) for k in range(N_CORES)]
    res = run_bass_kernel_spmd(nc, in_maps, core_ids=list(range(N_CORES)))

    out = np.empty((B, C, D, H, W), np.float32)
    for k in range(N_CORES):
        cs = slice(CH_PER_CORE * k, CH_PER_CORE * (k + 1))
        yc = (
            res.results[k]["out"]
            .astype(np.float32)
            .reshape(B, CH_PER_CORE, H, D, W)
        )
        out[:, cs] = yc.transpose(0, 1, 3, 2, 4)
    return out


# revision 3
# speedup vs baseline: 1.4883x; 1.4883x over previous
"""Trainium2 Bass kernel for DWConvBlock3D:
depthwise 3x3x3 conv (pad 1) + InstanceNorm3d + ReLU on x:(2,64,64,128,128) f32.

Strategy (8 NeuronCores, channel sharding => zero communication):
  - Each core owns 8 channels x 2 batches = 16 (b,c) "pairs".
  - Layout per pair: H=128 on SBUF partitions, (D,W) on the free dim, with
    host-side zero padding in both D (66) and W (130) so every matmul is
    uniform (no edge clipping).
  - The conv runs on TensorE as banded matmuls: a 128x128 banded matrix
    (3 diagonals = the kh taps) multiplies a (d,w)-shifted view of the x
    tile; shifts cover (kd,kw).  In fp16 that is 9 matmuls per 512-col
    chunk.  In fp8 mode we use DoubleRow perf mode (2 fused k-tile
    matmuls at 0.5 cycles/col): x is split hi+lo e4m3 (xh+xl ~ fp16
    precision) and the band w into bh+bl; the 28 products
      bh_t(xh+xl) [9], bl_t*xh [9], bl_c*xl [1], rest paired
    fit in 14 DoubleRow matmuls = 3584 PE cycles/chunk vs fp16's 4608.
  - PSUM: 2 bufs of 4 banks; each group of 4 chunks (2048 cols) is evicted
    in ONE ScalarE activation-copy (fp32->fp16, accum_out gives sum(y)).
  - sum(y^2) in one DVE pass (fp16 2x mode); cross-partition reduction via
    GpSimd partition_all_reduce; final normalize+ReLU is a single ScalarE
    activation with per-partition scale/bias; output DMA'd as fp16.
"""

import sys

if "/opt/trn_rl_repo" not in sys.path:
    sys.path.insert(0, "/opt/trn_rl_repo")

import numpy as np

B, C, D, H, W = 2, 64, 64, 128, 128
N_CORES = 8
CH_PER_CORE = C // N_CORES  # 8
N_PAIRS = B * CH_PER_CORE  # 16
DP = D + 2  # host-padded D
WP = W + 2  # host-padded W
FREE = D * W  # 8192 output cols per partition per pair
NV = D * H * W  # normalization element count per (b,c)
EPS = 1e-5
CD = 4  # d-slices per chunk (4*128 = 512 fp32 = 1 PSUM bank)
GROUP = 4  # chunks per PSUM buffer (4 banks, evicted as one 2048-col copy)
N_CHUNKS = D // CD  # 16
N_GROUPS = N_CHUNKS // GROUP  # 4
TAPS = [(kd, kw) for kd in range(3) for kw in range(3)]
SV = 2 * DP * WP  # fp8 x tile: partition stride (v=2 halves)
CONV_MODE = "fp16"  # "fp8" (DoubleRow) or "fp16"
# NOTE: measured on HW, DoubleRow fp8 matmuls stream output columns at the
# same 1 col/cycle as fp16 (the cost model's 0.5 cycles/row does not hold),
# so the 14-pass fp8 hi/lo scheme loses to the 9-pass fp16 conv.


def build_program(mode=CONV_MODE):
    import concourse.bacc as bacc
    import concourse.mybir as mybir
    from concourse import bass_isa
    from concourse.tile import TileContext
    from bass_rust import VecI64Pair

    f32 = mybir.dt.float32
    f16 = mybir.dt.float16
    f8 = mybir.dt.float8e4
    fp8 = mode == "fp8"
    nc = bacc.Bacc("TRN2", target_bir_lowering=False, debug=False, num_devices=N_CORES)

    if fp8:
        xs = nc.dram_tensor("xs", [N_PAIRS, H, 2, DP, WP], f8, kind="ExternalInput")
        bands = nc.dram_tensor(
            "bands", [H, CH_PER_CORE, 14, 2, H], f8, kind="ExternalInput"
        )
    else:
        xs = nc.dram_tensor("xs", [N_PAIRS, H, DP, WP], f16, kind="ExternalInput")
        bands = nc.dram_tensor(
            "bands", [H, CH_PER_CORE, 9, H], f16, kind="ExternalInput"
        )
    gb = nc.dram_tensor("gb", [128, 2 * N_PAIRS], f32, kind="ExternalInput")
    out = nc.dram_tensor("out", [N_PAIRS, H, FREE], f16, kind="ExternalOutput")

    mm_kw = dict(skip_group_check=True)
    if fp8:
        mm_kw["perf_mode"] = mybir.MatmulPerfMode.DoubleRow
        n_j = 14
    else:
        n_j = 9

    def rhs_view(xt, d0, j):
        """Moving operand for matmul j of the chunk at (true) depth d0."""
        if not fp8:
            kd, kw = TAPS[j]
            return xt[:, d0 + kd : d0 + kd + CD, kw : kw + W]
        if j < 9:  # (bh_t, bh_t) . (xh@t, xl@t)
            kd, kw = TAPS[j]
            return xt[:, :, d0 + kd : d0 + kd + CD, kw : kw + W]
        if j == 13:  # (bl_c, bl_c) . (xh@c, xl@c)
            return xt[:, :, d0 + 1 : d0 + 1 + CD, 1 : 1 + W]
        if j < 12:  # (bl_(0,kw), bl_(2,kw)) . (xh@(0,kw), xh@(2,kw))
            kw = j - 9
            delta = 2 * WP
        else:  # j == 12: (bl_(1,0), bl_(1,2)) . (xh@(1,0), xh@(1,2))
            kw = 0
            delta = 2
        kd0 = 0 if j < 12 else 1
        v = xt[:, :, d0 + kd0 : d0 + kd0 + CD, kw : kw + W].copy()
        ap = [tuple(p) for p in v.ap]
        ap[1] = (delta, 2)
        v.ap = VecI64Pair(ap)
        return v

    with TileContext(nc) as tc:
        with (
            tc.tile_pool(name="singles", bufs=1) as singles,
            tc.tile_pool(name="xp", bufs=3) as xpool,
            tc.tile_pool(name="yp", bufs=3) as ypool,
            tc.tile_pool(name="st", bufs=4) as stats,
            tc.tile_pool(name="psmm", bufs=2, space="PSUM") as psum_mm,
        ):
            if fp8:
                band_sb = singles.tile([H, CH_PER_CORE, 14, 2, H], f8)
            else:
                band_sb = singles.tile([H, CH_PER_CORE, 9, H], f16)
            gb_sb = singles.tile([128, 2 * N_PAIRS], f32)
            sq_sb = singles.tile([128, FREE], f16)  # y^2 scratch
            nc.sync.dma_start(out=gb_sb[:], in_=gb[:])

            for p in range(N_PAIRS):
                ci = p % CH_PER_CORE

                if fp8:
                    xt = xpool.tile([H, 2, DP, WP], f8, tag="xt")
                else:
                    xt = xpool.tile([H, DP, WP], f16, tag="xt")
                nc.sync.dma_start(out=xt[:], in_=xs[p])
                if p < CH_PER_CORE:
                    # just-in-time per-channel band load (keeps startup short)
                    nc.sync.dma_start(out=band_sb[:, ci], in_=bands[:, ci])

                y = ypool.tile([H, FREE], f16, tag="y")
                sums = stats.tile([128, N_GROUPS], f32, tag="sums")
                st2 = stats.tile([128, 2], f32, tag="st2")

                # ---- depthwise conv: banded matmuls, PSUM-accumulated
                for g in range(N_GROUPS):
                    ps = psum_mm.tile([128, GROUP * CD * W], f32, tag="mm",
                                      name=f"mm_{p}_{g}")
                    for j in range(n_j):
                        for c in range(GROUP):
                            d0 = (g * GROUP + c) * CD
                            if fp8:
                                lhsT = band_sb[:, ci, j]
                            else:
                                lhsT = band_sb[:, ci, j]
                            nc.tensor.matmul(
                                ps[:, c * CD * W : (c + 1) * CD * W],
                                lhsT,
                                rhs_view(xt, d0, j),
                                start=(j == 0),
                                stop=(j == n_j - 1),
                                **mm_kw,
                            )
                    # ---- evict group: PSUM -> y (fp16), sum(y) via accum_out
                    nc.scalar.activation(
                        out=y[:, g * GROUP * CD * W : (g + 1) * GROUP * CD * W],
                        in_=ps[:],
                        func=mybir.ActivationFunctionType.Copy,
                        accum_out=sums[:, g : g + 1],
                    )

                # ---- per-partition stats
                nc.vector.tensor_reduce(
                    out=st2[:, 0:1], in_=sums[:], axis=mybir.AxisListType.X,
                    op=mybir.AluOpType.add,
                )
                # sum(y^2) in one DVE pass (fp16 2x mode)
                nc.vector.scalar_tensor_tensor(
                    out=sq_sb[:], in0=y[:], scalar=1.0, in1=y[:],
                    op0=mybir.AluOpType.mult, op1=mybir.AluOpType.mult,
                    accum_out=st2[:, 1:2],
                )

                # ---- all-reduce across partitions (GpSimd) -> every partition
                # holds (sum, sumsq); the stats math then runs replicated
                ast = stats.tile([128, 2], f32, tag="ast")
                nc.gpsimd.partition_all_reduce(
                    ast[:], st2[:], 128, bass_isa.ReduceOp.add
                )

                sm = stats.tile([128, 10], f32, tag="sm")
                mean, ex2 = sm[:, 0:1], sm[:, 1:2]
                msq, vpe = sm[:, 2:3], sm[:, 3:4]
                std, r0 = sm[:, 4:5], sm[:, 5:6]
                t1, t2 = sm[:, 6:7], sm[:, 7:8]
                t4, rr = sm[:, 8:9], sm[:, 9:10]
                nc.vector.tensor_scalar_mul(mean, ast[:, 0:1], 1.0 / NV)
                nc.vector.tensor_scalar_mul(ex2, ast[:, 1:2], 1.0 / NV)
                nc.vector.tensor_mul(msq, mean, mean)
                nc.vector.tensor_sub(vpe, ex2, msq)
                nc.vector.tensor_scalar_add(vpe, vpe, EPS)
                nc.scalar.activation(std, vpe, mybir.ActivationFunctionType.Sqrt)
                nc.vector.reciprocal(r0, std)
                # one Newton step: r = r0*(1.5 - 0.5*vpe*r0^2)
                nc.vector.tensor_mul(t1, r0, r0)
                nc.vector.tensor_mul(t2, t1, vpe)
                nc.vector.tensor_scalar(
                    t4, t2, -0.5, 1.5, op0=mybir.AluOpType.mult, op1=mybir.AluOpType.add
                )
                nc.vector.tensor_mul(rr, r0, t4)

                sb2 = stats.tile([128, 2], f32, tag="sb2")
                sc, bi = sb2[:, 0:1], sb2[:, 1:2]
                # scale = gamma * rstd ; bias = beta - mean*scale
                nc.vector.tensor_mul(sc, rr, gb_sb[:, p : p + 1])
                nc.vector.tensor_mul(t1, mean, sc)
                nc.vector.tensor_sub(bi, gb_sb[:, N_PAIRS + p : N_PAIRS + p + 1], t1)

                # ---- fused normalize + ReLU (in place), then store.
                # split halves so the ScalarE apply overlaps the out-DMA
                hf = FREE // 2
                for h2 in range(2):
                    ysl = y[:, h2 * hf : (h2 + 1) * hf]
                    nc.scalar.activation(
                        out=ysl,
                        in_=ysl,
                        func=mybir.ActivationFunctionType.Relu,
                        scale=sc,
                        bias=bi,
                    )
                    nc.gpsimd.dma_start(
                        out=out[p][:, h2 * hf : (h2 + 1) * hf], in_=ysl
                    )

    nc.compile()
    return nc


_NC_CACHE = None


def _get_program():
    global _NC_CACHE
    if _NC_CACHE is None:
        _NC_CACHE = build_program()
    return _NC_CACHE


def _band_mats(w, c):
    """f32 band matrices [9, H, H] for channel c, tap order TAPS."""
    eye0 = np.eye(H, dtype=np.float32)
    eyep = np.eye(H, k=1, dtype=np.float32)  # B[h-1, h]: kh=0 tap
    eyem = np.eye(H, k=-1, dtype=np.float32)  # B[h+1, h]: kh=2 tap
    mats = np.empty((9, H, H), np.float32)
    for t, (kd, kw) in enumerate(TAPS):
        wk = w[c, 0, kd, :, kw]
        mats[t] = wk[0] * eyep + wk[1] * eye0 + wk[2] * eyem
    return mats


def make_core_inputs(x, w, gamma, beta, core, mode=CONV_MODE):
    import ml_dtypes

    f8 = ml_dtypes.float8_e4m3
    cs = slice(CH_PER_CORE * core, CH_PER_CORE * (core + 1))
    # (b, ci, d, h, w) -> (b, ci, h, d, w) -> (pair, h, d, w), pair = b*8+ci
    xt = (
        np.ascontiguousarray(x[:, cs].transpose(0, 1, 3, 2, 4))
        .reshape(N_PAIRS, H, D, W)
    )
    if mode == "fp8":
        xc = np.zeros((N_PAIRS, H, 2, DP, WP), f8)
        xh = xt.astype(f8)
        xl = (xt - xh.astype(np.float32)).astype(f8)
        xc[:, :, 0, 1 : D + 1, 1 : W + 1] = xh
        xc[:, :, 1, 1 : D + 1, 1 : W + 1] = xl
        bpk = np.zeros((H, CH_PER_CORE, 14, 2, H), np.float32)
        for ci in range(CH_PER_CORE):
            mats = _band_mats(w, CH_PER_CORE * core + ci)
            bh = mats.astype(f8).astype(np.float32)
            bl = (mats - bh).astype(f8).astype(np.float32)
            t_of = {t: i for i, t in enumerate(TAPS)}
            for j in range(9):
                bpk[:, ci, j, 0] = bh[j]
                bpk[:, ci, j, 1] = bh[j]
            for kw in range(3):
                bpk[:, ci, 9 + kw, 0] = bl[t_of[(0, kw)]]
                bpk[:, ci, 9 + kw, 1] = bl[t_of[(2, kw)]]
            bpk[:, ci, 12, 0] = bl[t_of[(1, 0)]]
            bpk[:, ci, 12, 1] = bl[t_of[(1, 2)]]
            bpk[:, ci, 13, 0] = bl[t_of[(1, 1)]]
            bpk[:, ci, 13, 1] = bl[t_of[(1, 1)]]
        band_arr = bpk.astype(f8)
    else:
        xc = np.zeros((N_PAIRS, H, DP, WP), np.float16)
        xc[:, :, 1 : D + 1, 1 : W + 1] = xt.astype(np.float16)
        bpk = np.zeros((H, CH_PER_CORE, 9, H), np.float32)
        for ci in range(CH_PER_CORE):
            bpk[:, ci] = _band_mats(w, CH_PER_CORE * core + ci).transpose(1, 0, 2)
        band_arr = bpk.astype(np.float16)
    gbv = np.broadcast_to(
        np.concatenate([np.tile(gamma[cs], B), np.tile(beta[cs], B)])
        .astype(np.float32)
        .reshape(1, 2 * N_PAIRS),
        (128, 2 * N_PAIRS),
    ).copy()
    return {"xs": xc, "bands": band_arr, "gb": gbv}


def kernel(x, w, gamma, beta):
    from concourse.bass_utils import run_bass_kernel_spmd

    x = np.asarray(x, dtype=np.float32)
    w = np.asarray(w, dtype=np.float32)
    gamma = np.asarray(gamma, dtype=np.float32)
    beta = np.asarray(beta, dtype=np.float32)

    nc = _get_program()
    in_maps = [make_core_inputs(x, w, gamma, beta, k) for k in range(N_CORES)]
    res = run_bass_kernel_spmd(nc, in_maps, core_ids=list(range(N_CORES)))

    out = np.empty((B, C, D, H, W), np.float32)
    for k in range(N_CORES):
        cs = slice(CH_PER_CORE * k, CH_PER_CORE * (k + 1))
        yc = (
            res.results[k]["out"]
            .astype(np.float32)
            .reshape(B, CH_PER_CORE, H, D, W)
        )
        out[:, cs] = yc.transpose(0, 1, 3, 2, 4)
    return out
